# revision 8
# baseline (speedup 1.0000x reference)
"""DeepConvNet Trainium2 kernel.

3x [Conv3x3(pad=1) -> ReLU -> MaxPool2x2] -> Linear, N=64, input 3x128x128.

Sharding: pure data parallel, 8 images per NeuronCore across 8 cores.

Per-core dataflow (activations bf16 in SBUF, fp32 PSUM accumulation):
  conv1: 3-vtap im2col, two 4-image groups CONCURRENT via PE row bands.
         K = 1 bias + 4 imgs x 3 ch x 3 vertical taps = 37 partitions per
         band (group 0 rows 0-36, group 1 rows 64-100); the 3 horizontal
         taps are PSUM-accumulated matmuls reading column-shifted windows
         (per-b weights in 3 lhsT column blocks, bias ones-row active only
         for b=0).  rhs = x replicated 3x (vertical shifts a*130), 2.4MB
         HBM traffic instead of the 9x/7.2MB full-tap replication; the DMA
         engines round-robin fairly across queued transfers, so small
         early y-chunks (8/8/16/32/64 rows) unblock conv1 k-blocks fast.
  conv2: DIRECT from pp1 -- 9 accumulated matmuls (K=64 = 2 imgs x 32 ch
         block-diagonal, M=128 = 2 imgs x 64 F) read tap-shifted windows
         of pp1 in place; the two image pairs of a group run CONCURRENTLY
         via PE row bands.  Bias+ReLU fold into the pool evacuation.
  conv3: 9 accumulated matmuls (K=64) per image; two images concurrent
         via row bands.  Bias+ReLU fold into the pool evacuation.
  pool:  PSUM can only feed one operand of a DVE op, so ScalarE copies
         even columns PSUM->SBUF (applying bias+ReLU where folded), DVE
         maxes odd PSUM columns against the copy, then the row-pair max
         alternates DVE/GpSimd (conv1's compressed window would otherwise
         be evacuation-bound on a single engine).
  fc:    256 accumulated matmuls (K=128 channels, one per spatial p),
         N = 8 images, M = 10 classes, 4-way column tiling.
"""

import os
import sys

import numpy as np

for _p in ("/opt/trn_rl_repo", "/root/.axon_site/_ro/trn_rl_repo"):
    if os.path.isdir(_p) and _p not in sys.path:
        sys.path.insert(0, _p)

import ml_dtypes

import concourse.bass as bass
import concourse.mybir as mybir
import concourse.tile as tile
from concourse import bacc
from concourse.bass_utils import run_bass_kernel_spmd

BF16 = mybir.dt.bfloat16
F32 = mybir.dt.float32
NPBF16 = ml_dtypes.bfloat16

N_CORES = 8
IMGS = 8          # images per core
GROUPS = 2        # conv1 image groups per core (4 imgs each)
G1 = 130          # conv1 padded width/height
W1ALLOC = 128 * G1  # conv1 rhs window length per partition
P1 = 66           # conv1 pooled padded grid (64 + 2)
P1F = 67 * 66     # pp1 alloc free size (one guard row)
P2 = 34           # conv2 pooled padded grid (32 + 2)
P2F = 34 * 34
WARMUP_MMS = 12   # 512-row junk matmuls: ramp the PE p-state until the
                  # first im2col chunk lands

# conv1 im2col y-chunks: chunk ci covers output rows [CHY[ci], CHY[ci+1])
# and gates the conv1 k-blocks that read those rows.
CHY = [0, 8, 16, 32, 64, 128]


def _build_nc(dbg=False):
    nc = bacc.Bacc("TRN2", target_bir_lowering=False, debug=False)

    xp = nc.dram_tensor("xp", [IMGS * 3 * G1 * G1], BF16, kind="ExternalInput")
    lhsT1 = nc.dram_tensor("lhsT1", [128, 384], BF16, kind="ExternalInput")
    wl2d = nc.dram_tensor("wl2d", [128, 1152], BF16, kind="ExternalInput")
    wrest = nc.dram_tensor("wrest", [128, 3712], BF16, kind="ExternalInput")
    wf32 = nc.dram_tensor("wf32", [128, 3], F32, kind="ExternalInput")
    ones_d = nc.dram_tensor("ones_d", [2 * W1ALLOC], BF16, kind="ExternalInput")
    scores = nc.dram_tensor("scores", [10, 8], F32, kind="ExternalOutput")

    Relu = mybir.ActivationFunctionType.Relu
    Copy = mybir.ActivationFunctionType.Copy
    MAX = mybir.AluOpType.max

    with tile.TileContext(nc) as tc:
        with (
            tc.tile_pool(name="wts", bufs=1) as wp,
            tc.tile_pool(name="rhs1", bufs=1) as rhs1p,
            tc.tile_pool(name="pp1", bufs=2) as pp1p,
            tc.tile_pool(name="pp2", bufs=4) as pp2p,
            tc.tile_pool(name="xall", bufs=1) as xallp,
            tc.tile_pool(name="tmp", bufs=6) as tmpp,
            tc.tile_pool(name="ps", bufs=4, space="PSUM") as psp,
        ):
            # ---- warmup: junk matmuls with no DMA dependency
            t_warm = wp.tile([128, 512], BF16)
            nc.gpsimd.memset(t_warm[:], 0)
            ps_warm = psp.tile([128, 512], F32, tag="ps", name="ps_warm")
            for _ in range(WARMUP_MMS):
                nc.tensor.matmul(
                    ps_warm[:], t_warm[:, 0:128], t_warm[:], start=True, stop=True
                )

            # ---- padded pool-output tiles: border memsets run early so
            # they never sit behind DMA waits in the gpsimd queue.
            # pp1 is ONE tile (group g at column offset g*P1F) so conv1's
            # final row-pair max covers both groups in one DVE op.
            pp1big = pp1p.tile([128, 2 * P1F], BF16, tag="pp1", name="pp1")
            pp1_tiles = [pp1big[:, g * P1F : (g + 1) * P1F] for g in range(GROUPS)]
            for g in range(GROUPS):
                pp1 = pp1_tiles[g]
                pv = pp1.rearrange("p (r q) -> p r q", q=P1)
                nc.gpsimd.memset(pp1[:, 0:P1], 0)
                nc.gpsimd.memset(pp1[:, 65 * P1 : P1F], 0)  # bottom + guard
                nc.gpsimd.memset(pv[:, 1:65, 0:1], 0)
                nc.gpsimd.memset(pv[:, 1:65, 65:66], 0)
            pp2_tiles = []
            for q in range(4):
                pp2 = pp2p.tile([128, P2F], BF16, tag="pp2", name=f"pp2_{q}")
                pv2 = pp2.rearrange("p (r q) -> p r q", q=P2)
                nc.gpsimd.memset(pp2[:, 0:P2], 0)
                nc.gpsimd.memset(pp2[:, 33 * P2 : P2F], 0)
                nc.gpsimd.memset(pv2[:, 1:33, 0:1], 0)
                nc.gpsimd.memset(pv2[:, 1:33, 33:34], 0)
                pp2_tiles.append(pp2)

            # ---- weight / constant loads.
            t_l1 = wp.tile([128, 384], BF16)
            nc.sync.dma_start(out=t_l1[:], in_=lhsT1.ap())
            t_wf32 = wp.tile([128, 3], F32)
            nc.scalar.dma_start(out=t_wf32[:], in_=wf32.ap())
            t_b3 = t_wf32[:, 0:1]
            t_bfc = t_wf32[0:10, 1:2]
            t_b2 = t_wf32[:, 2:3]

            rhs1 = rhs1p.tile([128, W1ALLOC], BF16, name="rhs1")
            r1pitch = rhs1.ap[0][0]
            # bias ones-rows at partitions 0 (group 0) and 64 (group 1)
            nc.scalar.dma_start(
                out=bass.AP(rhs1.tensor, rhs1.offset, [[64 * r1pitch, 2], [1, W1ALLOC]]),
                in_=bass.AP(ones_d, 0, [[W1ALLOC, 2], [1, W1ALLOC]]),
            )

            t_l2d = wp.tile([128, 1152], BF16)
            nc.scalar.dma_start(out=t_l2d[:], in_=wl2d.ap())

            # ---- im2col DMAs: per (y-chunk, vtap a) one DMA covering both
            # groups; partition (1 + imgc*3 + a) of band 64*g holds
            # xpad[img, c] vertically shifted by a rows (flat offset a*130).
            dmas = [nc.sync, nc.scalar, nc.gpsimd]
            for ci in range(len(CHY) - 1):
                c0 = CHY[ci] * G1
                wlen = (CHY[ci + 1] - CHY[ci]) * G1
                for a in range(3):
                    for g in range(2):
                        src = bass.AP(
                            xp,
                            g * 12 * G1 * G1 + a * G1 + c0,
                            [[G1 * G1, 12], [1, wlen]],
                        )
                        dst = bass.AP(
                            rhs1.tensor,
                            rhs1.offset + (64 * g + 1 + a) * r1pitch + c0,
                            [[3 * r1pitch, 12], [1, wlen]],
                        )
                        dmas[a].dma_start(out=dst, in_=src)

            # late weights (conv3 + fc), on sync after its taps
            t_wrest = wp.tile([128, 3712], BF16)
            nc.sync.dma_start(out=t_wrest[:], in_=wrest.ap())
            t_l3 = t_wrest[:, 0:1152]
            t_wfc = t_wrest[:, 1152:3712]

            if dbg:
                d_rhs1 = nc.dram_tensor(
                    "d_rhs1", [128, W1ALLOC], BF16, kind="ExternalOutput"
                )
                nc.sync.dma_start(out=d_rhs1.ap(), in_=rhs1[:])

            x_all = xallp.tile([128, 2048], BF16)

            def pool_psum_bias_relu(ps, out_ap, w, name, bias):
                """Pool with per-partition bias + ReLU folded into the two
                scalar-engine PSUM evacuations (bias/relu commute with max)."""
                psv = ps.rearrange("p (a two) -> p a two", two=2)
                cp = tmpp.tile([128, 512], F32, tag="tmpc", name=f"cpe_{name}")
                nc.scalar.activation(cp[:], psv[:, :, 0], Relu, bias=bias)
                cp2 = tmpp.tile([128, 512], F32, tag="tmpd", name=f"cpo_{name}")
                nc.scalar.activation(cp2[:], psv[:, :, 1], Relu, bias=bias)
                m1 = tmpp.tile([128, 512], BF16, tag="tmpm", name=f"m1_{name}")
                nc.vector.tensor_max(m1[:], cp2[:], cp[:])
                tv = m1.rearrange("p (y two x) -> p y two x", two=2, x=w // 2)
                nc.vector.tensor_max(out_ap, tv[:, :, 0, :], tv[:, :, 1, :])

            # =======================  conv1  =======================
            # Both groups stream concurrently: group g occupies PE rows
            # 64g..64g+36.  Per (k, h): 3 accumulated matmuls per group,
            # matmul b reading the window shifted b columns right with the
            # b-th lhsT column block.
            rhs1v = rhs1.rearrange("p (y x) -> p y x", x=G1)
            for k in range(16):
                ps_g = [
                    psp.tile([128, 1024], F32, tag="ps", name=f"ps1_{k}_{gg}")
                    for gg in range(2)
                ]
                for h in range(2):
                    y0 = k * 8 + h * 4
                    for b in range(3):
                        for g in range(2):
                            nc.tensor.matmul(
                                ps_g[g][:, h * 512 : (h + 1) * 512],
                                t_l1[64 * g : 64 * g + 37, b * 128 : (b + 1) * 128],
                                rhs1v[64 * g : 64 * g + 37, y0 : y0 + 4, b : b + 128],
                                start=(b == 0),
                                stop=(b == 2),
                            )
                # pool evacuation: per group ScalarE relu-evacs the even
                # psum columns (relu commutes into the max chain), DVE
                # maxes the odd columns against it; the final row-pair max
                # covers BOTH groups in one DVE op (merged m1 / pp1).
                m1k = tmpp.tile([128, 1024], BF16, tag="tmpm1", name=f"m1_{k}")
                for g in range(2):
                    psv = ps_g[g].rearrange("p (a two) -> p a two", two=2)
                    cp = tmpp.tile([128, 512], F32, tag="tmpc", name=f"cp1_{k}_{g}")
                    nc.scalar.activation(cp[:], psv[:, :, 0], Relu)
                    nc.vector.tensor_max(
                        m1k[:, 512 * g : 512 * (g + 1)], psv[:, :, 1], cp[:]
                    )
                tv = m1k.rearrange("p (g y two x) -> p g y two x", g=2, y=4, two=2, x=64)
                pvb = pp1big.rearrange("p (g r q) -> p g r q", g=2, q=P1)
                Y0 = k * 4
                nc.vector.tensor_max(
                    pvb[:, :, Y0 + 1 : Y0 + 5, 1:65],
                    tv[:, :, :, 0, :],
                    tv[:, :, :, 1, :],
                )

            # =======================  conv2 (direct from pp1)  =======================
            def conv2_group(g):
                """Both pairs of group g run concurrently: pair A = imgs
                4g+0,4g+1 (pp1 rows 0-63, PE rows 0-63), pair B = imgs
                4g+2,4g+3 (rows 64-127).  9 accumulated taps read
                tap-shifted pp1 windows in place."""
                pv = pp1_tiles[g].rearrange("p (r q) -> p r q", q=P1)
                for k in range(4):
                    ps_ab = [
                        psp.tile([128, 1024], F32, tag="ps", name=f"ps2_{g}_{k}_{jj}")
                        for jj in range(2)
                    ]
                    for h in range(2):
                        Y0 = k * 16 + h * 8
                        for t in range(9):
                            a, b = divmod(t, 3)
                            for j in range(2):  # pair A rows 0-63, pair B 64-127
                                nc.tensor.matmul(
                                    ps_ab[j][:, h * 512 : (h + 1) * 512],
                                    t_l2d[64 * j : 64 * j + 64, t * 128 : (t + 1) * 128],
                                    pv[64 * j : 64 * j + 64, Y0 + a : Y0 + a + 8, b : b + 64],
                                    start=(t == 0),
                                    stop=(t == 8),
                                )
                    for j in range(2):
                        q = 2 * g + j
                        pv2 = pp2_tiles[q].rearrange("p (r q) -> p r q", q=P2)
                        Y0 = k * 8
                        pool_psum_bias_relu(
                            ps_ab[j], pv2[:, Y0 + 1 : Y0 + 9, 1:33], 64,
                            f"c2_{q}_{k}", t_b2,
                        )

            def conv3_pair(q):
                pv2 = pp2_tiles[q].rearrange("p (r q) -> p r q", q=P2)
                ps_ab = [
                    psp.tile([128, 1024], F32, tag="ps", name=f"ps3_{q}_{jj}")
                    for jj in range(2)
                ]
                for h in range(2):
                    Y0 = h * 16
                    for t in range(9):
                        a, b = divmod(t, 3)
                        for j in range(2):  # img A (rows 0-63), img B (rows 64-127)
                            nc.tensor.matmul(
                                ps_ab[j][:, h * 512 : (h + 1) * 512],
                                t_l3[64 * j : 64 * j + 64, t * 128 : (t + 1) * 128],
                                pv2[64 * j : 64 * j + 64, Y0 + a : Y0 + a + 16, b : b + 32],
                                start=(t == 0),
                                stop=(t == 8),
                            )
                for j in range(2):
                    img = 2 * q + j
                    xv = x_all.rearrange("p (i q) -> p i q", q=256)
                    ov = xv[:, img, :].rearrange("p (y x) -> p y x", x=16)
                    pool_psum_bias_relu(ps_ab[j], ov, 32, f"c3_{q}_{j}", t_b3)

            conv2_group(0)
            conv3_pair(0)
            conv3_pair(1)
            conv2_group(1)
            conv3_pair(2)
            conv3_pair(3)

            if dbg:
                d_pp1 = nc.dram_tensor("d_pp1", [128, P1F], BF16, kind="ExternalOutput")
                nc.sync.dma_start(out=d_pp1.ap(), in_=pp1_tiles[0][:])
                d_pp2 = nc.dram_tensor("d_pp2", [128, P2F], BF16, kind="ExternalOutput")
                nc.sync.dma_start(out=d_pp2.ap(), in_=pp2_tiles[0][:])
                d_xall = nc.dram_tensor("d_xall", [128, 2048], BF16, kind="ExternalOutput")
                nc.sync.dma_start(out=d_xall.ap(), in_=x_all[:])

            # =======================  fc  =======================
            ps_fc = psp.tile([128, 8], F32, tag="ps", name="ps_fc")
            xv = x_all.rearrange("p (i q) -> p i q", q=256)
            for p in range(256):
                cg = p % 4
                nc.tensor.matmul(
                    ps_fc[32 * cg : 32 * cg + 10, :],
                    t_wfc[:, 10 * p : 10 * p + 10],
                    xv[:, :, p],
                    start=(p < 4),
                    stop=(p >= 252),
                    tile_position=(0, 32 * cg),
                )
            sc0 = wp.tile([10, 8], F32)
            nc.scalar.activation(sc0[:], ps_fc[0:10, :], Copy)
            sc1 = wp.tile([10, 8], F32)
            nc.vector.tensor_add(sc1[:], ps_fc[32:42, :], sc0[:])
            sc2 = wp.tile([10, 8], F32)
            nc.vector.tensor_add(sc2[:], ps_fc[64:74, :], sc1[:])
            sc3 = wp.tile([10, 8], F32)
            nc.vector.tensor_add(sc3[:], ps_fc[96:106, :], sc2[:])
            sc = wp.tile([10, 8], F32)
            nc.scalar.activation(
                sc[:], sc3[:], mybir.ActivationFunctionType.Identity, bias=t_bfc
            )
            nc.sync.dma_start(out=scores.ap(), in_=sc[:])

    nc.compile()
    return nc


def _prep_weights(w1, b1, w2, b2, w3, b3, w_fc, b_fc):
    """Host-side weight rearrangement (shared across cores)."""
    # conv1 lhsT: column block b holds tap column b; row 1 + imgc*3 + a,
    # col m = img*32 + f.  Row 0 carries the bias (rhs ones-row), active
    # only in the b=0 block.  Band 64.. duplicates rows 0-36 for group 1.
    l1 = np.zeros((128, 384), np.float32)
    for b in range(3):
        for a in range(3):
            for img in range(4):
                for c in range(3):
                    r = 1 + (img * 3 + c) * 3 + a
                    l1[r, b * 128 + img * 32 : b * 128 + img * 32 + 32] = w1[:, c, a, b]
    l1[0, 0:128] = np.tile(np.asarray(b1, np.float32), 4)
    l1[64:101, :] = l1[0:37, :]
    # conv2 direct: per tap t a [128, 128] block: rows 0-31 (img-even ch)
    # -> cols 0-63 (img-even F), rows 32-63 (img-odd ch) -> cols 64-127;
    # rows 64-127 duplicate rows 0-63 (pair B at PE rows 64-127).
    l2d = np.zeros((128, 9 * 128), np.float32)
    for t in range(9):
        a, b = divmod(t, 3)
        blk = w2[:, :, a, b].T  # [c=32, f=64]
        l2d[0:32, t * 128 : t * 128 + 64] = blk
        l2d[32:64, t * 128 + 64 : t * 128 + 128] = blk
    l2d[64:128, :] = l2d[0:64, :]
    # conv3: rows c (dup at 64+c), col block t
    l3 = np.zeros((128, 9 * 128), np.float32)
    for t in range(9):
        a, b = divmod(t, 3)
        blk = w3[:, :, a, b].T  # [c=64, f=128]
        l3[0:64, t * 128 : (t + 1) * 128] = blk
        l3[64:128, t * 128 : (t + 1) * 128] = blk
    # fc: w_fc[c*256 + p, cls] -> wfc[c, p*10 + cls]
    wf = np.ascontiguousarray(w_fc.reshape(128, 256, 10).reshape(128, 2560))
    wrest = np.concatenate([l3, wf], axis=1)
    wf32 = np.zeros((128, 3), np.float32)
    wf32[:, 0] = np.asarray(b3, np.float32)
    wf32[0:10, 1] = np.asarray(b_fc, np.float32)
    wf32[:, 2] = np.tile(np.asarray(b2, np.float32), 2)
    return {
        "lhsT1": l1.astype(NPBF16),
        "wl2d": l2d.astype(NPBF16),
        "wrest": wrest.astype(NPBF16),
        "wf32": wf32,
        "ones_d": np.ones(2 * W1ALLOC, NPBF16),
    }


_NC_CACHE = {}


def get_nc():
    if "nc" not in _NC_CACHE:
        _NC_CACHE["nc"] = _build_nc()
    return _NC_CACHE["nc"]


def kernel(x, w1, b1, w2, b2, w3, b3, w_fc, b_fc, **run_kwargs):
    x = np.asarray(x, np.float32)
    wts = _prep_weights(
        np.asarray(w1, np.float32), np.asarray(b1, np.float32),
        np.asarray(w2, np.float32), np.asarray(b2, np.float32),
        np.asarray(w3, np.float32), np.asarray(b3, np.float32),
        np.asarray(w_fc, np.float32), np.asarray(b_fc, np.float32),
    )
    xpad = np.pad(x, ((0, 0), (0, 0), (1, 1), (1, 1))).astype(NPBF16)
    in_maps = []
    for core in range(N_CORES):
        m = dict(wts)
        m["xp"] = np.ascontiguousarray(xpad[core * IMGS : (core + 1) * IMGS]).reshape(-1)
        in_maps.append(m)

    nc = get_nc()
    res = run_bass_kernel_spmd(nc, in_maps, core_ids=list(range(N_CORES)), **run_kwargs)
    out = np.concatenate([r["scores"].T for r in res.results], axis=0)
    kernel.last_results = res
    return out.astype(np.float32)


# revision 9
# speedup vs baseline: 1.1977x; 1.1977x over previous
"""DeepConvNet Trainium2 kernel.

3x [Conv3x3(pad=1) -> ReLU -> MaxPool2x2] -> Linear, N=64, input 3x128x128.

Sharding: pure data parallel, 8 images per NeuronCore across 8 cores.

Per-core dataflow (activations bf16 in SBUF, fp32 PSUM accumulation):
  conv1: 3-vtap im2col, two 4-image groups CONCURRENT via PE row bands.
         K = 1 bias + 4 imgs x 3 ch x 3 vertical taps = 37 partitions per
         band (group 0 rows 0-36, group 1 rows 64-100); the 3 horizontal
         taps are PSUM-accumulated matmuls reading column-shifted windows
         (per-b weights in 3 lhsT column blocks, bias ones-row active only
         for b=0).  rhs = x replicated 3x (vertical shifts a*130), 2.4MB
         HBM traffic instead of the 9x/7.2MB full-tap replication; the DMA
         engines round-robin fairly across queued transfers, so small
         early y-chunks (8/8/16/32/64 rows) unblock conv1 k-blocks fast.
  conv2: DIRECT from pp1 -- 9 accumulated matmuls (K=64 = 2 imgs x 32 ch
         block-diagonal, M=128 = 2 imgs x 64 F) read tap-shifted windows
         of pp1 in place; the two image pairs of a group run CONCURRENTLY
         via PE row bands.  Bias+ReLU fold into the pool evacuation.
  conv3: 9 accumulated matmuls (K=64) per image; two images concurrent
         via row bands.  Bias+ReLU fold into the pool evacuation.
  pool:  PSUM can only feed one operand of a DVE op, so ScalarE copies
         even columns PSUM->SBUF (applying bias+ReLU where folded), DVE
         maxes odd PSUM columns against the copy, then the row-pair max
         alternates DVE/GpSimd (conv1's compressed window would otherwise
         be evacuation-bound on a single engine).
  fc:    256 accumulated matmuls (K=128 channels, one per spatial p),
         N = 8 images, M = 10 classes, 4-way column tiling.
"""

import os
import sys

import numpy as np

for _p in ("/opt/trn_rl_repo", "/root/.axon_site/_ro/trn_rl_repo"):
    if os.path.isdir(_p) and _p not in sys.path:
        sys.path.insert(0, _p)

import ml_dtypes

import concourse.bass as bass
import concourse.mybir as mybir
import concourse.tile as tile
from concourse import bacc
from concourse.bass_utils import run_bass_kernel_spmd

BF16 = mybir.dt.bfloat16
F32 = mybir.dt.float32
NPBF16 = ml_dtypes.bfloat16

N_CORES = 8
IMGS = 8          # images per core
GROUPS = 2        # conv1 image groups per core (4 imgs each)
G1 = 130          # conv1 padded width/height
W1ALLOC = 128 * G1  # conv1 rhs window length per partition
P1 = 66           # conv1 pooled padded grid (64 + 2)
P1F = 67 * 66     # pp1 alloc free size (one guard row)
P2 = 34           # conv2 pooled padded grid (32 + 2)
P2F = 34 * 34
WARMUP_MMS = 12   # 512-row junk matmuls: ramp the PE p-state until the
                  # first im2col chunk lands

# conv1 im2col y-chunks: chunk ci covers output rows [CHY[ci], CHY[ci+1])
# and gates the conv1 k-blocks that read those rows.
CHY = [0, 8, 16, 32, 64, 128]


def _build_nc(dbg=False):
    nc = bacc.Bacc("TRN2", target_bir_lowering=False, debug=False)

    xp = nc.dram_tensor("xp", [IMGS * 3 * G1 * G1], BF16, kind="ExternalInput")
    lhsT1 = nc.dram_tensor("lhsT1", [128, 384], BF16, kind="ExternalInput")
    wl2d = nc.dram_tensor("wl2d", [128, 1152], BF16, kind="ExternalInput")
    wrest = nc.dram_tensor("wrest", [128, 3712], BF16, kind="ExternalInput")
    wf32 = nc.dram_tensor("wf32", [128, 3], F32, kind="ExternalInput")
    ones_d = nc.dram_tensor("ones_d", [2 * W1ALLOC], BF16, kind="ExternalInput")
    scores = nc.dram_tensor("scores", [10, 8], F32, kind="ExternalOutput")

    Relu = mybir.ActivationFunctionType.Relu
    Copy = mybir.ActivationFunctionType.Copy
    MAX = mybir.AluOpType.max

    with tile.TileContext(nc) as tc:
        with (
            tc.tile_pool(name="wts", bufs=1) as wp,
            tc.tile_pool(name="rhs1", bufs=1) as rhs1p,
            tc.tile_pool(name="pp1", bufs=2) as pp1p,
            tc.tile_pool(name="pp2", bufs=4) as pp2p,
            tc.tile_pool(name="xall", bufs=1) as xallp,
            tc.tile_pool(name="tmp", bufs=6) as tmpp,
            tc.tile_pool(name="ps", bufs=4, space="PSUM") as psp,
        ):
            # ---- warmup: junk matmuls with no DMA dependency
            t_warm = wp.tile([128, 512], BF16)
            nc.gpsimd.memset(t_warm[:], 0)
            ps_warm = psp.tile([128, 512], F32, tag="ps", name="ps_warm")
            for _ in range(WARMUP_MMS):
                nc.tensor.matmul(
                    ps_warm[:], t_warm[:, 0:128], t_warm[:], start=True, stop=True
                )

            # ---- early loads: conv1's gating data goes FIRST on each ring
            # (the DMA engines round-robin fairly across everything queued,
            # so anything issued before the im2col steals its bandwidth).
            t_l1 = wp.tile([128, 384], BF16)
            nc.sync.dma_start(out=t_l1[:], in_=lhsT1.ap())
            t_wf32 = wp.tile([128, 3], F32)
            nc.scalar.dma_start(out=t_wf32[:], in_=wf32.ap())
            t_b3 = t_wf32[:, 0:1]
            t_bfc = t_wf32[0:10, 1:2]
            t_b2 = t_wf32[:, 2:3]

            rhs1 = rhs1p.tile([128, W1ALLOC], BF16, name="rhs1")
            r1pitch = rhs1.ap[0][0]
            # bias ones-rows at partitions 0 (group 0) and 64 (group 1)
            nc.scalar.dma_start(
                out=bass.AP(rhs1.tensor, rhs1.offset, [[64 * r1pitch, 2], [1, W1ALLOC]]),
                in_=bass.AP(ones_d, 0, [[W1ALLOC, 2], [1, W1ALLOC]]),
            )

            # ---- im2col DMAs: per (y-chunk, vtap a, group) one DMA;
            # partition 64g + 1 + imgc*3 + a holds xpad[img, c] vertically
            # shifted by a rows (flat offset a*130).
            dmas = [nc.sync, nc.scalar, nc.gpsimd]
            for ci in range(len(CHY) - 1):
                c0 = CHY[ci] * G1
                wlen = (CHY[ci + 1] - CHY[ci]) * G1
                for a in range(3):
                    for g in range(2):
                        src = bass.AP(
                            xp,
                            g * 12 * G1 * G1 + a * G1 + c0,
                            [[G1 * G1, 12], [1, wlen]],
                        )
                        dst = bass.AP(
                            rhs1.tensor,
                            rhs1.offset + (64 * g + 1 + a) * r1pitch + c0,
                            [[3 * r1pitch, 12], [1, wlen]],
                        )
                        dmas[a].dma_start(out=dst, in_=src)

            # late weights: wl2d needed at conv2 (~mid-kernel), wrest
            # (conv3 + fc) even later -- both AFTER the im2col on their ring
            t_l2d = wp.tile([128, 1152], BF16)
            nc.scalar.dma_start(out=t_l2d[:], in_=wl2d.ap())
            t_wrest = wp.tile([128, 3712], BF16)
            nc.sync.dma_start(out=t_wrest[:], in_=wrest.ap())
            t_l3 = t_wrest[:, 0:1152]
            t_wfc = t_wrest[:, 1152:3712]

            # ---- padded pool-output tiles: border memsets go AFTER the
            # gpsimd ring's DMA issues (they're not needed until the first
            # pool evac).  pp1 is ONE tile (group g at column offset
            # g*P1F) so conv1's final row-pair max covers both groups in
            # one DVE op.
            pp1big = pp1p.tile([128, 2 * P1F], BF16, tag="pp1", name="pp1")
            pp1_tiles = [pp1big[:, g * P1F : (g + 1) * P1F] for g in range(GROUPS)]
            for g in range(GROUPS):
                pp1 = pp1_tiles[g]
                pv = pp1.rearrange("p (r q) -> p r q", q=P1)
                nc.gpsimd.memset(pp1[:, 0:P1], 0)
                nc.gpsimd.memset(pp1[:, 65 * P1 : P1F], 0)  # bottom + guard
                nc.gpsimd.memset(pv[:, 1:65, 0:1], 0)
                nc.gpsimd.memset(pv[:, 1:65, 65:66], 0)
            pp2_tiles = []
            for q in range(4):
                pp2 = pp2p.tile([128, P2F], BF16, tag="pp2", name=f"pp2_{q}")
                pv2 = pp2.rearrange("p (r q) -> p r q", q=P2)
                nc.gpsimd.memset(pp2[:, 0:P2], 0)
                nc.gpsimd.memset(pp2[:, 33 * P2 : P2F], 0)
                nc.gpsimd.memset(pv2[:, 1:33, 0:1], 0)
                nc.gpsimd.memset(pv2[:, 1:33, 33:34], 0)
                pp2_tiles.append(pp2)

            if dbg:
                d_rhs1 = nc.dram_tensor(
                    "d_rhs1", [128, W1ALLOC], BF16, kind="ExternalOutput"
                )
                nc.sync.dma_start(out=d_rhs1.ap(), in_=rhs1[:])

            x_all = xallp.tile([128, 2048], BF16)

            def pool_psum_bias_relu(ps, out_ap, w, name, bias):
                """Pool with per-partition bias + ReLU folded into the two
                scalar-engine PSUM evacuations (bias/relu commute with max)."""
                psv = ps.rearrange("p (a two) -> p a two", two=2)
                cp = tmpp.tile([128, 512], F32, tag="tmpc", name=f"cpe_{name}")
                nc.scalar.activation(cp[:], psv[:, :, 0], Relu, bias=bias)
                cp2 = tmpp.tile([128, 512], F32, tag="tmpd", name=f"cpo_{name}")
                nc.scalar.activation(cp2[:], psv[:, :, 1], Relu, bias=bias)
                m1 = tmpp.tile([128, 512], BF16, tag="tmpm", name=f"m1_{name}")
                nc.vector.tensor_max(m1[:], cp2[:], cp[:])
                tv = m1.rearrange("p (y two x) -> p y two x", two=2, x=w // 2)
                nc.vector.tensor_max(out_ap, tv[:, :, 0, :], tv[:, :, 1, :])

            # =======================  conv1  =======================
            # Both groups stream concurrently: group g occupies PE rows
            # 64g..64g+36.  Per (k, h): 3 accumulated matmuls per group,
            # matmul b reading the window shifted b columns right with the
            # b-th lhsT column block.
            rhs1v = rhs1.rearrange("p (y x) -> p y x", x=G1)
            for k in range(16):
                ps_g = [
                    psp.tile([128, 1024], F32, tag="ps", name=f"ps1_{k}_{gg}")
                    for gg in range(2)
                ]
                for h in range(2):
                    y0 = k * 8 + h * 4
                    for b in range(3):
                        for g in range(2):
                            nc.tensor.matmul(
                                ps_g[g][:, h * 512 : (h + 1) * 512],
                                t_l1[64 * g : 64 * g + 37, b * 128 : (b + 1) * 128],
                                rhs1v[64 * g : 64 * g + 37, y0 : y0 + 4, b : b + 128],
                                start=(b == 0),
                                stop=(b == 2),
                            )
                # pool evacuation: per group ScalarE relu-evacs the even
                # psum columns (relu commutes into the max chain), DVE
                # maxes the odd columns against it; the final row-pair max
                # covers BOTH groups in one DVE op (merged m1 / pp1).
                m1k = tmpp.tile([128, 1024], BF16, tag="tmpm1", name=f"m1_{k}")
                for g in range(2):
                    psv = ps_g[g].rearrange("p (a two) -> p a two", two=2)
                    cp = tmpp.tile([128, 512], F32, tag="tmpc", name=f"cp1_{k}_{g}")
                    nc.scalar.activation(cp[:], psv[:, :, 0], Relu)
                    nc.vector.tensor_max(
                        m1k[:, 512 * g : 512 * (g + 1)], psv[:, :, 1], cp[:]
                    )
                tv = m1k.rearrange("p (g y two x) -> p g y two x", g=2, y=4, two=2, x=64)
                pvb = pp1big.rearrange("p (g r q) -> p g r q", g=2, q=P1)
                Y0 = k * 4
                nc.vector.tensor_max(
                    pvb[:, :, Y0 + 1 : Y0 + 5, 1:65],
                    tv[:, :, :, 0, :],
                    tv[:, :, :, 1, :],
                )

            # =======================  conv2 (direct from pp1)  =======================
            def conv2_group(g):
                """Both pairs of group g run concurrently: pair A = imgs
                4g+0,4g+1 (pp1 rows 0-63, PE rows 0-63), pair B = imgs
                4g+2,4g+3 (rows 64-127).  9 accumulated taps read
                tap-shifted pp1 windows in place."""
                pv = pp1_tiles[g].rearrange("p (r q) -> p r q", q=P1)
                for k in range(4):
                    ps_ab = [
                        psp.tile([128, 1024], F32, tag="ps", name=f"ps2_{g}_{k}_{jj}")
                        for jj in range(2)
                    ]
                    for h in range(2):
                        Y0 = k * 16 + h * 8
                        for t in range(9):
                            a, b = divmod(t, 3)
                            for j in range(2):  # pair A rows 0-63, pair B 64-127
                                nc.tensor.matmul(
                                    ps_ab[j][:, h * 512 : (h + 1) * 512],
                                    t_l2d[64 * j : 64 * j + 64, t * 128 : (t + 1) * 128],
                                    pv[64 * j : 64 * j + 64, Y0 + a : Y0 + a + 8, b : b + 64],
                                    start=(t == 0),
                                    stop=(t == 8),
                                )
                    for j in range(2):
                        q = 2 * g + j
                        pv2 = pp2_tiles[q].rearrange("p (r q) -> p r q", q=P2)
                        Y0 = k * 8
                        pool_psum_bias_relu(
                            ps_ab[j], pv2[:, Y0 + 1 : Y0 + 9, 1:33], 64,
                            f"c2_{q}_{k}", t_b2,
                        )

            def conv3_pair(q):
                pv2 = pp2_tiles[q].rearrange("p (r q) -> p r q", q=P2)
                ps_ab = [
                    psp.tile([128, 1024], F32, tag="ps", name=f"ps3_{q}_{jj}")
                    for jj in range(2)
                ]
                for h in range(2):
                    Y0 = h * 16
                    for t in range(9):
                        a, b = divmod(t, 3)
                        for j in range(2):  # img A (rows 0-63), img B (rows 64-127)
                            nc.tensor.matmul(
                                ps_ab[j][:, h * 512 : (h + 1) * 512],
                                t_l3[64 * j : 64 * j + 64, t * 128 : (t + 1) * 128],
                                pv2[64 * j : 64 * j + 64, Y0 + a : Y0 + a + 16, b : b + 32],
                                start=(t == 0),
                                stop=(t == 8),
                            )
                for j in range(2):
                    img = 2 * q + j
                    xv = x_all.rearrange("p (i q) -> p i q", q=256)
                    ov = xv[:, img, :].rearrange("p (y x) -> p y x", x=16)
                    pool_psum_bias_relu(ps_ab[j], ov, 32, f"c3_{q}_{j}", t_b3)

            conv2_group(0)
            conv3_pair(0)
            conv3_pair(1)
            conv2_group(1)
            conv3_pair(2)
            conv3_pair(3)

            if dbg:
                d_pp1 = nc.dram_tensor("d_pp1", [128, P1F], BF16, kind="ExternalOutput")
                nc.sync.dma_start(out=d_pp1.ap(), in_=pp1_tiles[0][:])
                d_pp2 = nc.dram_tensor("d_pp2", [128, P2F], BF16, kind="ExternalOutput")
                nc.sync.dma_start(out=d_pp2.ap(), in_=pp2_tiles[0][:])
                d_xall = nc.dram_tensor("d_xall", [128, 2048], BF16, kind="ExternalOutput")
                nc.sync.dma_start(out=d_xall.ap(), in_=x_all[:])

            # =======================  fc  =======================
            ps_fc = psp.tile([128, 8], F32, tag="ps", name="ps_fc")
            xv = x_all.rearrange("p (i q) -> p i q", q=256)
            for p in range(256):
                cg = p % 4
                nc.tensor.matmul(
                    ps_fc[32 * cg : 32 * cg + 10, :],
                    t_wfc[:, 10 * p : 10 * p + 10],
                    xv[:, :, p],
                    start=(p < 4),
                    stop=(p >= 252),
                    tile_position=(0, 32 * cg),
                )
            sc0 = wp.tile([10, 8], F32)
            nc.scalar.activation(sc0[:], ps_fc[0:10, :], Copy)
            sc1 = wp.tile([10, 8], F32)
            nc.vector.tensor_add(sc1[:], ps_fc[32:42, :], sc0[:])
            sc2 = wp.tile([10, 8], F32)
            nc.vector.tensor_add(sc2[:], ps_fc[64:74, :], sc1[:])
            sc3 = wp.tile([10, 8], F32)
            nc.vector.tensor_add(sc3[:], ps_fc[96:106, :], sc2[:])
            sc = wp.tile([10, 8], F32)
            nc.scalar.activation(
                sc[:], sc3[:], mybir.ActivationFunctionType.Identity, bias=t_bfc
            )
            nc.sync.dma_start(out=scores.ap(), in_=sc[:])

    nc.compile()
    return nc


def _prep_weights(w1, b1, w2, b2, w3, b3, w_fc, b_fc):
    """Host-side weight rearrangement (shared across cores)."""
    # conv1 lhsT: column block b holds tap column b; row 1 + imgc*3 + a,
    # col m = img*32 + f.  Row 0 carries the bias (rhs ones-row), active
    # only in the b=0 block.  Band 64.. duplicates rows 0-36 for group 1.
    l1 = np.zeros((128, 384), np.float32)
    for b in range(3):
        for a in range(3):
            for img in range(4):
                for c in range(3):
                    r = 1 + (img * 3 + c) * 3 + a
                    l1[r, b * 128 + img * 32 : b * 128 + img * 32 + 32] = w1[:, c, a, b]
    l1[0, 0:128] = np.tile(np.asarray(b1, np.float32), 4)
    l1[64:101, :] = l1[0:37, :]
    # conv2 direct: per tap t a [128, 128] block: rows 0-31 (img-even ch)
    # -> cols 0-63 (img-even F), rows 32-63 (img-odd ch) -> cols 64-127;
    # rows 64-127 duplicate rows 0-63 (pair B at PE rows 64-127).
    l2d = np.zeros((128, 9 * 128), np.float32)
    for t in range(9):
        a, b = divmod(t, 3)
        blk = w2[:, :, a, b].T  # [c=32, f=64]
        l2d[0:32, t * 128 : t * 128 + 64] = blk
        l2d[32:64, t * 128 + 64 : t * 128 + 128] = blk
    l2d[64:128, :] = l2d[0:64, :]
    # conv3: rows c (dup at 64+c), col block t
    l3 = np.zeros((128, 9 * 128), np.float32)
    for t in range(9):
        a, b = divmod(t, 3)
        blk = w3[:, :, a, b].T  # [c=64, f=128]
        l3[0:64, t * 128 : (t + 1) * 128] = blk
        l3[64:128, t * 128 : (t + 1) * 128] = blk
    # fc: w_fc[c*256 + p, cls] -> wfc[c, p*10 + cls]
    wf = np.ascontiguousarray(w_fc.reshape(128, 256, 10).reshape(128, 2560))
    wrest = np.concatenate([l3, wf], axis=1)
    wf32 = np.zeros((128, 3), np.float32)
    wf32[:, 0] = np.asarray(b3, np.float32)
    wf32[0:10, 1] = np.asarray(b_fc, np.float32)
    wf32[:, 2] = np.tile(np.asarray(b2, np.float32), 2)
    return {
        "lhsT1": l1.astype(NPBF16),
        "wl2d": l2d.astype(NPBF16),
        "wrest": wrest.astype(NPBF16),
        "wf32": wf32,
        "ones_d": np.ones(2 * W1ALLOC, NPBF16),
    }


_NC_CACHE = {}


def get_nc():
    if "nc" not in _NC_CACHE:
        _NC_CACHE["nc"] = _build_nc()
    return _NC_CACHE["nc"]


def kernel(x, w1, b1, w2, b2, w3, b3, w_fc, b_fc, **run_kwargs):
    x = np.asarray(x, np.float32)
    wts = _prep_weights(
        np.asarray(w1, np.float32), np.asarray(b1, np.float32),
        np.asarray(w2, np.float32), np.asarray(b2, np.float32),
        np.asarray(w3, np.float32), np.asarray(b3, np.float32),
        np.asarray(w_fc, np.float32), np.asarray(b_fc, np.float32),
    )
    xpad = np.pad(x, ((0, 0), (0, 0), (1, 1), (1, 1))).astype(NPBF16)
    in_maps = []
    for core in range(N_CORES):
        m = dict(wts)
        m["xp"] = np.ascontiguousarray(xpad[core * IMGS : (core + 1) * IMGS]).reshape(-1)
        in_maps.append(m)

    nc = get_nc()
    res = run_bass_kernel_spmd(nc, in_maps, core_ids=list(range(N_CORES)), **run_kwargs)
    out = np.concatenate([r["scores"].T for r in res.results], axis=0)
    kernel.last_results = res
    return out.astype(np.float32)


# revision 14
# speedup vs baseline: 1.2017x; 1.0033x over previous
"""DeepConvNet Trainium2 kernel.

3x [Conv3x3(pad=1) -> ReLU -> MaxPool2x2] -> Linear, N=64, input 3x128x128.

Sharding: pure data parallel, 8 images per NeuronCore across 8 cores.

Per-core dataflow (activations bf16 in SBUF, fp32 PSUM accumulation):
  conv1: 3-vtap im2col, two 4-image groups CONCURRENT via PE row bands.
         K = 1 bias + 4 imgs x 3 ch x 3 vertical taps = 37 partitions per
         band (group 0 rows 0-36, group 1 rows 64-100); the 3 horizontal
         taps are PSUM-accumulated matmuls reading column-shifted windows
         (per-b weights in 3 lhsT column blocks, bias ones-row active only
         for b=0).  rhs = x replicated 3x (vertical shifts a*130), 2.4MB
         HBM traffic instead of the 9x/7.2MB full-tap replication; the DMA
         engines round-robin fairly across queued transfers, so small
         early y-chunks (8/8/16/32/64 rows) unblock conv1 k-blocks fast.
  conv2: DIRECT from pp1 -- 9 accumulated matmuls (K=64 = 2 imgs x 32 ch
         block-diagonal, M=128 = 2 imgs x 64 F) read tap-shifted windows
         of pp1 in place; the two image pairs of a group run CONCURRENTLY
         via PE row bands.  Bias+ReLU fold into the pool evacuation.
  conv3: 9 accumulated matmuls (K=64) per image; two images concurrent
         via row bands.  Bias+ReLU fold into the pool evacuation.
  pool:  PSUM can only feed one operand of a DVE op, so ScalarE copies
         even columns PSUM->SBUF (applying bias+ReLU where folded), DVE
         maxes odd PSUM columns against the copy, then the row-pair max
         alternates DVE/GpSimd (conv1's compressed window would otherwise
         be evacuation-bound on a single engine).
  fc:    256 accumulated matmuls (K=128 channels, one per spatial p),
         N = 8 images, M = 10 classes, 4-way column tiling.
"""

import os
import sys

import numpy as np

for _p in ("/opt/trn_rl_repo", "/root/.axon_site/_ro/trn_rl_repo"):
    if os.path.isdir(_p) and _p not in sys.path:
        sys.path.insert(0, _p)

import ml_dtypes

import concourse.bass as bass
import concourse.mybir as mybir
import concourse.tile as tile
from concourse import bacc
from concourse.bass_utils import run_bass_kernel_spmd

BF16 = mybir.dt.bfloat16
F32 = mybir.dt.float32
NPBF16 = ml_dtypes.bfloat16

N_CORES = 8
IMGS = 8          # images per core
GROUPS = 2        # conv1 image groups per core (4 imgs each)
G1 = 130          # conv1 padded width/height
W1ALLOC = 128 * G1  # conv1 rhs window length per partition
P1 = 66           # conv1 pooled padded grid (64 + 2)
P1F = 67 * 66     # pp1 alloc free size (one guard row)
P2 = 34           # conv2 pooled padded grid (32 + 2)
P2F = 34 * 34
WARMUP_MMS = 12   # 512-row junk matmuls: ramp the PE p-state until the
                  # first im2col chunk lands

# conv1 im2col y-chunks: chunk ci covers output rows [CHY[ci], CHY[ci+1])
# and gates the conv1 k-blocks that read those rows.
CHY = [0, 8, 16, 32, 64, 128]


def _build_nc(dbg=False):
    nc = bacc.Bacc("TRN2", target_bir_lowering=False, debug=False)

    # xp: per group 21 planes (12 real img,ch planes + dup of the first 9,
    # used to pad conv1's K to 64 rows/band with half-weights -- the HAM
    # clock gate only un-throttles under high PE row occupancy)
    xp = nc.dram_tensor("xp", [2 * 21 * G1 * G1], BF16, kind="ExternalInput")
    lhsT1 = nc.dram_tensor("lhsT1", [128, 384], BF16, kind="ExternalInput")
    wl2d = nc.dram_tensor("wl2d", [128, 1152], BF16, kind="ExternalInput")
    wrest = nc.dram_tensor("wrest", [128, 3712], BF16, kind="ExternalInput")
    wf32 = nc.dram_tensor("wf32", [128, 3], F32, kind="ExternalInput")
    ones_d = nc.dram_tensor("ones_d", [2 * W1ALLOC], BF16, kind="ExternalInput")
    scores = nc.dram_tensor("scores", [10, 8], F32, kind="ExternalOutput")

    Relu = mybir.ActivationFunctionType.Relu
    Copy = mybir.ActivationFunctionType.Copy
    MAX = mybir.AluOpType.max

    with tile.TileContext(nc) as tc:
        with (
            tc.tile_pool(name="wts", bufs=1) as wp,
            tc.tile_pool(name="rhs1", bufs=1) as rhs1p,
            tc.tile_pool(name="pp1", bufs=2) as pp1p,
            tc.tile_pool(name="pp2", bufs=4) as pp2p,
            tc.tile_pool(name="xall", bufs=1) as xallp,
            tc.tile_pool(name="tmp", bufs=6) as tmpp,
            tc.tile_pool(name="ps", bufs=4, space="PSUM") as psp,
        ):
            # ---- warmup: junk matmuls with no DMA dependency
            t_warm = wp.tile([128, 512], BF16)
            nc.gpsimd.memset(t_warm[:], 0)
            ps_warm = psp.tile([128, 512], F32, tag="ps", name="ps_warm")
            for _ in range(WARMUP_MMS):
                nc.tensor.matmul(
                    ps_warm[:], t_warm[:, 0:128], t_warm[:], start=True, stop=True
                )

            # ---- early loads: conv1's gating data goes FIRST on each ring
            # (the DMA engines round-robin fairly across everything queued,
            # so anything issued before the im2col steals its bandwidth).
            t_l1 = wp.tile([128, 384], BF16)
            nc.sync.dma_start(out=t_l1[:], in_=lhsT1.ap())
            t_wf32 = wp.tile([128, 3], F32)
            nc.scalar.dma_start(out=t_wf32[:], in_=wf32.ap())
            t_b3 = t_wf32[:, 0:1]
            t_bfc = t_wf32[0:10, 1:2]
            t_b2 = t_wf32[:, 2:3]

            rhs1 = rhs1p.tile([128, W1ALLOC], BF16, name="rhs1")
            r1pitch = rhs1.ap[0][0]
            # bias ones-rows at partitions 0 (group 0) and 64 (group 1)
            nc.scalar.dma_start(
                out=bass.AP(rhs1.tensor, rhs1.offset, [[64 * r1pitch, 2], [1, W1ALLOC]]),
                in_=bass.AP(ones_d, 0, [[W1ALLOC, 2], [1, W1ALLOC]]),
            )

            # ---- im2col DMAs: per (y-chunk, vtap a, group) one DMA;
            # partition 64g + 1 + i*3 + a (i = 0..20: 12 real planes + 9
            # dup planes, contiguous in xp) holds plane i vertically
            # shifted by a rows (flat offset a*130).  Fills the whole
            # 64-row band: rows 1..63 data + row 0 bias ones.
            dmas = [nc.sync, nc.scalar, nc.gpsimd]
            for ci in range(len(CHY) - 1):
                c0 = CHY[ci] * G1
                wlen = (CHY[ci + 1] - CHY[ci]) * G1
                for a in range(3):
                    for g in range(2):
                        src = bass.AP(
                            xp,
                            g * 21 * G1 * G1 + a * G1 + c0,
                            [[G1 * G1, 21], [1, wlen]],
                        )
                        dst = bass.AP(
                            rhs1.tensor,
                            rhs1.offset + (64 * g + 1 + a) * r1pitch + c0,
                            [[3 * r1pitch, 21], [1, wlen]],
                        )
                        # late a=1 chunks ride gpsimd so the scalar queue
                        # frees up for the first pool evacuations
                        q = nc.gpsimd if (a == 1 and ci >= 3) else dmas[a]
                        q.dma_start(out=dst, in_=src)

            # late weights: wl2d needed at conv2 (~mid-kernel), wrest
            # (conv3 + fc) even later -- both AFTER the im2col on their ring
            t_l2d = wp.tile([128, 1152], BF16)
            nc.scalar.dma_start(out=t_l2d[:], in_=wl2d.ap())
            t_wrest = wp.tile([128, 3712], BF16)
            nc.sync.dma_start(out=t_wrest[:], in_=wrest.ap())
            t_l3 = t_wrest[:, 0:1152]
            t_wfc = t_wrest[:, 1152:3712]

            # ---- padded pool-output tiles: border memsets go AFTER the
            # gpsimd ring's DMA issues (they're not needed until the first
            # pool evac).  pp1 is ONE tile (group g at column offset
            # g*P1F) so conv1's final row-pair max covers both groups in
            # one DVE op.
            pp1big = pp1p.tile([128, 2 * P1F], BF16, tag="pp1", name="pp1")
            pp1_tiles = [pp1big[:, g * P1F : (g + 1) * P1F] for g in range(GROUPS)]
            for g in range(GROUPS):
                pp1 = pp1_tiles[g]
                pv = pp1.rearrange("p (r q) -> p r q", q=P1)
                nc.gpsimd.memset(pp1[:, 0:P1], 0)
                nc.gpsimd.memset(pp1[:, 65 * P1 : P1F], 0)  # bottom + guard
                nc.gpsimd.memset(pv[:, 1:65, 0:1], 0)
                nc.gpsimd.memset(pv[:, 1:65, 65:66], 0)
            pp2_tiles = []
            for q in range(4):
                pp2 = pp2p.tile([128, P2F], BF16, tag="pp2", name=f"pp2_{q}")
                pv2 = pp2.rearrange("p (r q) -> p r q", q=P2)
                nc.gpsimd.memset(pp2[:, 0:P2], 0)
                nc.gpsimd.memset(pp2[:, 33 * P2 : P2F], 0)
                nc.gpsimd.memset(pv2[:, 1:33, 0:1], 0)
                nc.gpsimd.memset(pv2[:, 1:33, 33:34], 0)
                pp2_tiles.append(pp2)

            if dbg:
                d_rhs1 = nc.dram_tensor(
                    "d_rhs1", [128, W1ALLOC], BF16, kind="ExternalOutput"
                )
                nc.sync.dma_start(out=d_rhs1.ap(), in_=rhs1[:])

            x_all = xallp.tile([128, 2048], BF16)

            def pool_psum_bias_relu(ps, out_ap, w, name, bias):
                """Pool with per-partition bias + ReLU folded into the two
                scalar-engine PSUM evacuations (bias/relu commute with max)."""
                psv = ps.rearrange("p (a two) -> p a two", two=2)
                cp = tmpp.tile([128, 512], F32, tag="tmpc", name=f"cpe_{name}")
                nc.scalar.activation(cp[:], psv[:, :, 0], Relu, bias=bias)
                cp2 = tmpp.tile([128, 512], F32, tag="tmpd", name=f"cpo_{name}")
                nc.scalar.activation(cp2[:], psv[:, :, 1], Relu, bias=bias)
                m1 = tmpp.tile([128, 512], BF16, tag="tmpm", name=f"m1_{name}")
                nc.vector.tensor_max(m1[:], cp2[:], cp[:])
                tv = m1.rearrange("p (y two x) -> p y two x", two=2, x=w // 2)
                nc.vector.tensor_max(out_ap, tv[:, :, 0, :], tv[:, :, 1, :])

            # =======================  conv1  =======================
            # Both groups stream concurrently: group g occupies PE rows
            # 64g..64g+36.  Per (k, h): 3 accumulated matmuls per group,
            # matmul b reading the window shifted b columns right with the
            # b-th lhsT column block.
            rhs1v = rhs1.rearrange("p (y x) -> p y x", x=G1)
            for k in range(16):
                ps_g = [
                    psp.tile([128, 1024], F32, tag="ps", name=f"ps1_{k}_{gg}")
                    for gg in range(2)
                ]
                for h in range(2):
                    y0 = k * 8 + h * 4
                    for b in range(3):
                        for g in range(2):
                            nc.tensor.matmul(
                                ps_g[g][:, h * 512 : (h + 1) * 512],
                                t_l1[64 * g : 64 * g + 64, b * 128 : (b + 1) * 128],
                                rhs1v[64 * g : 64 * g + 64, y0 : y0 + 4, b : b + 128],
                                start=(b == 0),
                                stop=(b == 2),
                            )
                # pool evacuation: per group ScalarE relu-evacs the even
                # psum columns (relu commutes into the max chain), DVE
                # maxes the odd columns against it; the final row-pair max
                # covers BOTH groups in one DVE op (merged m1 / pp1).
                m1k = tmpp.tile([128, 1024], BF16, tag="tmpm1", name=f"m1_{k}")
                for g in range(2):
                    psv = ps_g[g].rearrange("p (a two) -> p a two", two=2)
                    cp = tmpp.tile([128, 512], F32, tag="tmpc", name=f"cp1_{k}_{g}")
                    nc.scalar.activation(cp[:], psv[:, :, 0], Relu)
                    nc.vector.tensor_max(
                        m1k[:, 512 * g : 512 * (g + 1)], psv[:, :, 1], cp[:]
                    )
                tv = m1k.rearrange("p (g y two x) -> p g y two x", g=2, y=4, two=2, x=64)
                pvb = pp1big.rearrange("p (g r q) -> p g r q", g=2, q=P1)
                Y0 = k * 4
                nc.vector.tensor_max(
                    pvb[:, :, Y0 + 1 : Y0 + 5, 1:65],
                    tv[:, :, :, 0, :],
                    tv[:, :, :, 1, :],
                )

            # =======================  conv2 (direct from pp1)  =======================
            def conv2_group(g):
                """Both pairs of group g run concurrently: pair A = imgs
                4g+0,4g+1 (pp1 rows 0-63, PE rows 0-63), pair B = imgs
                4g+2,4g+3 (rows 64-127).  9 accumulated taps read
                tap-shifted pp1 windows in place."""
                pv = pp1_tiles[g].rearrange("p (r q) -> p r q", q=P1)
                for k in range(4):
                    ps_ab = [
                        psp.tile([128, 1024], F32, tag="ps", name=f"ps2_{g}_{k}_{jj}")
                        for jj in range(2)
                    ]
                    for h in range(2):
                        Y0 = k * 16 + h * 8
                        for t in range(9):
                            a, b = divmod(t, 3)
                            for j in range(2):  # pair A rows 0-63, pair B 64-127
                                nc.tensor.matmul(
                                    ps_ab[j][:, h * 512 : (h + 1) * 512],
                                    t_l2d[64 * j : 64 * j + 64, t * 128 : (t + 1) * 128],
                                    pv[64 * j : 64 * j + 64, Y0 + a : Y0 + a + 8, b : b + 64],
                                    start=(t == 0),
                                    stop=(t == 8),
                                )
                    for j in range(2):
                        q = 2 * g + j
                        pv2 = pp2_tiles[q].rearrange("p (r q) -> p r q", q=P2)
                        Y0 = k * 8
                        pool_psum_bias_relu(
                            ps_ab[j], pv2[:, Y0 + 1 : Y0 + 9, 1:33], 64,
                            f"c2_{q}_{k}", t_b2,
                        )

            def conv3_pair(q):
                pv2 = pp2_tiles[q].rearrange("p (r q) -> p r q", q=P2)
                ps_ab = [
                    psp.tile([128, 1024], F32, tag="ps", name=f"ps3_{q}_{jj}")
                    for jj in range(2)
                ]
                for h in range(2):
                    Y0 = h * 16
                    for t in range(9):
                        a, b = divmod(t, 3)
                        for j in range(2):  # img A (rows 0-63), img B (rows 64-127)
                            nc.tensor.matmul(
                                ps_ab[j][:, h * 512 : (h + 1) * 512],
                                t_l3[64 * j : 64 * j + 64, t * 128 : (t + 1) * 128],
                                pv2[64 * j : 64 * j + 64, Y0 + a : Y0 + a + 16, b : b + 32],
                                start=(t == 0),
                                stop=(t == 8),
                            )
                for j in range(2):
                    img = 2 * q + j
                    xv = x_all.rearrange("p (i q) -> p i q", q=256)
                    ov = xv[:, img, :].rearrange("p (y x) -> p y x", x=16)
                    pool_psum_bias_relu(ps_ab[j], ov, 32, f"c3_{q}_{j}", t_b3)

            conv2_group(0)
            conv3_pair(0)
            conv3_pair(1)
            conv2_group(1)
            conv3_pair(2)
            conv3_pair(3)

            if dbg:
                d_pp1 = nc.dram_tensor("d_pp1", [128, P1F], BF16, kind="ExternalOutput")
                nc.sync.dma_start(out=d_pp1.ap(), in_=pp1_tiles[0][:])
                d_pp2 = nc.dram_tensor("d_pp2", [128, P2F], BF16, kind="ExternalOutput")
                nc.sync.dma_start(out=d_pp2.ap(), in_=pp2_tiles[0][:])
                d_xall = nc.dram_tensor("d_xall", [128, 2048], BF16, kind="ExternalOutput")
                nc.sync.dma_start(out=d_xall.ap(), in_=x_all[:])

            # =======================  fc  =======================
            ps_fc = psp.tile([128, 8], F32, tag="ps", name="ps_fc")
            xv = x_all.rearrange("p (i q) -> p i q", q=256)
            for p in range(256):
                cg = p % 4
                nc.tensor.matmul(
                    ps_fc[32 * cg : 32 * cg + 10, :],
                    t_wfc[:, 10 * p : 10 * p + 10],
                    xv[:, :, p],
                    start=(p < 4),
                    stop=(p >= 252),
                    tile_position=(0, 32 * cg),
                )
            sc0 = wp.tile([10, 8], F32)
            nc.scalar.activation(sc0[:], ps_fc[0:10, :], Copy)
            sc1 = wp.tile([10, 8], F32)
            nc.vector.tensor_add(sc1[:], ps_fc[32:42, :], sc0[:])
            sc2 = wp.tile([10, 8], F32)
            nc.vector.tensor_add(sc2[:], ps_fc[64:74, :], sc1[:])
            sc3 = wp.tile([10, 8], F32)
            nc.vector.tensor_add(sc3[:], ps_fc[96:106, :], sc2[:])
            sc = wp.tile([10, 8], F32)
            nc.scalar.activation(
                sc[:], sc3[:], mybir.ActivationFunctionType.Identity, bias=t_bfc
            )
            nc.sync.dma_start(out=scores.ap(), in_=sc[:])

    nc.compile()
    return nc


def _prep_weights(w1, b1, w2, b2, w3, b3, w_fc, b_fc):
    """Host-side weight rearrangement (shared across cores)."""
    # conv1 lhsT: column block b holds tap column b; row 1 + i*3 + a for
    # plane i (i = 0..11 real imgc, i = 12..20 dup of imgc 0..8 at half
    # weight on both copies -- pads K to 64 so the HAM clock un-throttles),
    # col m = img*32 + f.  Row 0 carries the bias (rhs ones-row), active
    # only in the b=0 block.  Band 64.. duplicates rows 0-63 for group 1.
    l1 = np.zeros((128, 384), np.float32)
    for b in range(3):
        for a in range(3):
            for i in range(21):
                imgc = i if i < 12 else i - 12
                img, c = divmod(imgc, 3)
                scale = 0.5 if imgc < 9 else 1.0
                r = 1 + i * 3 + a
                l1[r, b * 128 + img * 32 : b * 128 + img * 32 + 32] = (
                    w1[:, c, a, b] * scale
                )
    l1[0, 0:128] = np.tile(np.asarray(b1, np.float32), 4)
    l1[64:128, :] = l1[0:64, :]
    # conv2 direct: per tap t a [128, 128] block: rows 0-31 (img-even ch)
    # -> cols 0-63 (img-even F), rows 32-63 (img-odd ch) -> cols 64-127;
    # rows 64-127 duplicate rows 0-63 (pair B at PE rows 64-127).
    l2d = np.zeros((128, 9 * 128), np.float32)
    for t in range(9):
        a, b = divmod(t, 3)
        blk = w2[:, :, a, b].T  # [c=32, f=64]
        l2d[0:32, t * 128 : t * 128 + 64] = blk
        l2d[32:64, t * 128 + 64 : t * 128 + 128] = blk
    l2d[64:128, :] = l2d[0:64, :]
    # conv3: rows c (dup at 64+c), col block t
    l3 = np.zeros((128, 9 * 128), np.float32)
    for t in range(9):
        a, b = divmod(t, 3)
        blk = w3[:, :, a, b].T  # [c=64, f=128]
        l3[0:64, t * 128 : (t + 1) * 128] = blk
        l3[64:128, t * 128 : (t + 1) * 128] = blk
    # fc: w_fc[c*256 + p, cls] -> wfc[c, p*10 + cls]
    wf = np.ascontiguousarray(w_fc.reshape(128, 256, 10).reshape(128, 2560))
    wrest = np.concatenate([l3, wf], axis=1)
    wf32 = np.zeros((128, 3), np.float32)
    wf32[:, 0] = np.asarray(b3, np.float32)
    wf32[0:10, 1] = np.asarray(b_fc, np.float32)
    wf32[:, 2] = np.tile(np.asarray(b2, np.float32), 2)
    return {
        "lhsT1": l1.astype(NPBF16),
        "wl2d": l2d.astype(NPBF16),
        "wrest": wrest.astype(NPBF16),
        "wf32": wf32,
        "ones_d": np.ones(2 * W1ALLOC, NPBF16),
    }


_NC_CACHE = {}


def get_nc():
    if "nc" not in _NC_CACHE:
        _NC_CACHE["nc"] = _build_nc()
    return _NC_CACHE["nc"]


def kernel(x, w1, b1, w2, b2, w3, b3, w_fc, b_fc, **run_kwargs):
    x = np.asarray(x, np.float32)
    wts = _prep_weights(
        np.asarray(w1, np.float32), np.asarray(b1, np.float32),
        np.asarray(w2, np.float32), np.asarray(b2, np.float32),
        np.asarray(w3, np.float32), np.asarray(b3, np.float32),
        np.asarray(w_fc, np.float32), np.asarray(b_fc, np.float32),
    )
    xpad = np.pad(x, ((0, 0), (0, 0), (1, 1), (1, 1))).astype(NPBF16)
    in_maps = []
    for core in range(N_CORES):
        planes = np.ascontiguousarray(
            xpad[core * IMGS : (core + 1) * IMGS]
        ).reshape(2, 12, G1 * G1)
        m = dict(wts)
        # per group: 12 real planes + dup of the first 9 (conv1 K padding)
        m["xp"] = np.concatenate(
            [np.concatenate([planes[g], planes[g, 0:9]]) for g in range(2)]
        ).reshape(-1)
        in_maps.append(m)

    nc = get_nc()
    res = run_bass_kernel_spmd(nc, in_maps, core_ids=list(range(N_CORES)), **run_kwargs)
    out = np.concatenate([r["scores"].T for r in res.results], axis=0)
    kernel.last_results = res
    return out.astype(np.float32)
